# revision 13
# baseline (speedup 1.0000x reference)
"""Trainium2 Bass kernel for AttentionRNN (GRU cell step + Bahdanau attention).

Contract: kernel(**inputs) takes the FULL unsharded inputs (B=64) and returns
the full outputs (h_new, ctx, align). Internally the batch is sharded across
8 NeuronCores (8 samples per core); weights are replicated. No cross-device
communication is needed.

Math (per sample):
  x  = [memory, context]                      (768,)
  gi = x @ W_ih.T + b_ih ; gh = h @ W_hh.T + b_hh
  r = sig(gi_r+gh_r); z = sig(gi_z+gh_z); n = tanh(gi_n + r*gh_n)
  h' = (1-z)*n + z*h
  pa = A @ Wa.T + ba                          (T, H)
  s  = tanh(h'@Wq.T + bq + pa) @ v            (T,)
  al = softmax(s); ctx = al @ A               (H,)

Device layout choices:
  - All linear-layer weights are fed host-pre-transposed (W.T) so the
    contraction dim lands on SBUF partitions; matmuls run as
    out[m,n] = sum_p lhsT[p,m] * rhs[p,n].
  - pa is computed TRANSPOSED (features j on partitions, time r on free dim):
    lhsT = Wa.T tile, rhs = A.T tile. A.T is produced on-chip by PE-mode
    transposes of the natural-layout A (which is itself needed for ctx).
  - tanh(pq + bq + ba + pa) is fused into one ScalarE activation with a
    per-partition bias; scores then contract j via a v-column matmul, which
    leaves scores on the free dim -> cheap free-dim softmax.
  - exp(scores) is PE-transposed into [t,1] columns; ctx_unnorm = exp @ A via
    matmuls against natural A; both ctx and align are scaled by 1/sum(exp).
  - Everything that feeds a matmul is typed float32r end-to-end (DRAM tensors,
    SBUF tiles): 1 cycle/row at N>=256 vs 4 for fp32, and the BIR verifier
    requires fp32r matmul inputs to be *produced* as fp32r. Bytes are
    identical to fp32 on the host side.
"""

import os
import sys

for _p in ("/opt/trn_rl_repo",):
    if _p not in sys.path and os.path.isdir(_p):
        sys.path.insert(0, _p)

from contextlib import ExitStack

import numpy as np

import concourse.bass as bass
import concourse.tile as tile
from concourse import bacc, mybir
from concourse.bass_utils import run_bass_kernel_spmd
from concourse.masks import make_identity

# Problem shapes (hardcoded per contract).
B, T, H, M = 64, 1024, 512, 256
NCORES = 8
BL = B // NCORES  # samples per core
KX = M + H        # GRU input width (768)
H3 = 3 * H
FP = mybir.dt.float32
FPR = mybir.dt.float32r
AF = mybir.ActivationFunctionType
ALU = mybir.AluOpType
AX = mybir.AxisListType


def build_kernel():
    """Builds and compiles the per-core Bass program. Returns nc."""
    nc = bacc.Bacc(
        "TRN2",
        target_bir_lowering=False,
        debug=False,
        enable_asserts=False,
        num_devices=NCORES,
    )

    # DRAM I/O (per core). float32r tensors carry plain fp32 bytes.
    a = nc.dram_tensor("a", [BL, T, H], FPR, kind="ExternalInput").ap()
    xT = nc.dram_tensor("xT", [KX, BL], FPR, kind="ExternalInput").ap()
    hT = nc.dram_tensor("hT", [H, BL], FPR, kind="ExternalInput").ap()
    h_nat = nc.dram_tensor("h_nat", [BL, H], FP, kind="ExternalInput").ap()
    wihT = nc.dram_tensor("wihT", [KX, H3], FPR, kind="ExternalInput").ap()
    whhT = nc.dram_tensor("whhT", [H, H3], FPR, kind="ExternalInput").ap()
    wqT = nc.dram_tensor("wqT", [H, H], FPR, kind="ExternalInput").ap()
    waT = nc.dram_tensor("waT", [H, H], FPR, kind="ExternalInput").ap()
    bih = nc.dram_tensor("bih", [BL, H3], FP, kind="ExternalInput").ap()
    bhh = nc.dram_tensor("bhh", [BL, H3], FP, kind="ExternalInput").ap()
    bqaT = nc.dram_tensor("bqaT", [128, 4], FP, kind="ExternalInput").ap()
    vT = nc.dram_tensor("vT", [128, 4], FPR, kind="ExternalInput").ap()

    h_new_o = nc.dram_tensor("h_new_o", [BL, H], FP, kind="ExternalOutput").ap()
    ctx_o = nc.dram_tensor("ctx_o", [BL, H], FP, kind="ExternalOutput").ap()
    align_o = nc.dram_tensor("align_o", [BL, T], FP, kind="ExternalOutput").ap()

    with tile.TileContext(nc) as tc, ExitStack() as ctx:
        _body(ctx, tc, a, xT, hT, h_nat, wihT, whhT, wqT, waT, bih, bhh,
              bqaT, vT, h_new_o, ctx_o, align_o)

    nc.compile()
    return nc


def _body(ctx, tc, a, xT, hT, h_nat, wihT, whhT, wqT, waT, bih, bhh,
          bqaT, vT, h_new_o, ctx_o, align_o):
    nc = tc.nc

    # ----- persistent tiles -----
    per = ctx.enter_context(tc.tile_pool(name="per", bufs=1))
    ident = per.tile([128, 128], FP)
    make_identity(nc, ident)
    # fp32r copy of the identity so A-block transposes run in fp32r mode
    # (1.5 cycles/row instead of 2).
    identR = per.tile([128, 128], FPR)
    nc.vector.tensor_copy(identR, ident)

    waT_sb = per.tile([128, 4, H], FPR)
    nc.sync.dma_start(out=waT_sb, in_=waT.rearrange("(c p) j -> p c j", p=128))
    vT_sb = per.tile([128, 4], FPR)
    nc.sync.dma_start(out=vT_sb, in_=vT)
    bqaT_sb = per.tile([128, 4], FP)
    nc.sync.dma_start(out=bqaT_sb, in_=bqaT)
    # tanh bias per (j, sample): pq.T + (bq+ba).T, filled during GRU phase.
    pqT_sb = per.tile([128, 4, BL], FP)
    hnew_sb = per.tile([BL, H], FP)

    # Annotations pools are entered before the GRU phase so the first
    # samples' DMAs overlap GRU compute.
    apool = ctx.enter_context(tc.tile_pool(name="apool", bufs=3))
    atpool = ctx.enter_context(tc.tile_pool(name="atpool", bufs=2))
    thpool = ctx.enter_context(tc.tile_pool(name="thpool", bufs=4))
    smpool = ctx.enter_context(tc.tile_pool(name="smpool", bufs=2))
    a_r = a.rearrange("s (c p) d -> s p c d", p=128)
    a_tiles = {}

    def load_a(s):
        t = apool.tile([128, T // 128, H], FPR, tag="a", name=f"a{s}")
        nc.sync.dma_start(out=t, in_=a_r[s])
        a_tiles[s] = t

    def transpose_a(s):
        aT_s = atpool.tile([128, 4, T], FPR, tag="at", name=f"at{s}")
        a_s = a_tiles[s]
        for kc in range(4):
            for half in range(2):
                tp = ps_tr.tile([128, 512], FPR, tag="tr")
                for i in range(4):
                    tci = half * 4 + i
                    nc.tensor.transpose(
                        tp[:, i * 128:(i + 1) * 128],
                        a_s[:, tci, kc * 128:(kc + 1) * 128],
                        identR)
                nc.any.tensor_copy(aT_s[:, kc, half * 512:(half + 1) * 512], tp)
        return aT_s

    # =======================  Phase 1: GRU  =======================
    with (
        tc.tile_pool(name="gw", bufs=3) as gw,
        tc.tile_pool(name="gs", bufs=1) as gs,
        tc.tile_pool(name="gps", bufs=1, space="PSUM") as gps,
    ):
        xT_sb = gs.tile([128, KX // 128, BL], FPR)
        nc.sync.dma_start(out=xT_sb, in_=xT.rearrange("(c p) s -> p c s", p=128))
        hT_sb = gs.tile([128, H // 128, BL], FPR)
        nc.sync.dma_start(out=hT_sb, in_=hT.rearrange("(c p) s -> p c s", p=128))
        h_sb = gs.tile([BL, H], FP)
        nc.sync.dma_start(out=h_sb, in_=h_nat)
        bih_sb = gs.tile([BL, H3], FP)
        nc.sync.dma_start(out=bih_sb, in_=bih)
        bhh_sb = gs.tile([BL, H3], FP)
        nc.sync.dma_start(out=bhh_sb, in_=bhh)
        wqT_sb = gs.tile([128, 4, H], FPR)
        nc.sync.dma_start(out=wqT_sb, in_=wqT.rearrange("(c p) j -> p c j", p=128))

        # gates: rz_ps accumulates x@Wih.T + h@Whh.T for the r,z thirds;
        # the n third is kept split (i_n, h_n) because r gates only h_n.
        rz_ps = gps.tile([BL, 2, H], FP)     # 2 banks
        in_ps = gps.tile([BL, H], FP)        # 1 bank
        hn_ps = gps.tile([BL, H], FP)        # 1 bank
        nkx = KX // 128
        nh = H // 128
        for kc in range(nkx):
            wt = gw.tile([128, H3], FPR, tag="w")
            nc.sync.dma_start(out=wt, in_=wihT[kc * 128:(kc + 1) * 128, :])
            lhs = xT_sb[:, kc]
            for jt in range(2):
                nc.tensor.matmul(rz_ps[:, jt], lhs, wt[:, jt * H:(jt + 1) * H],
                                 start=(kc == 0), stop=False)
            nc.tensor.matmul(in_ps, lhs, wt[:, 2 * H:3 * H],
                             start=(kc == 0), stop=(kc == nkx - 1))
        for kc in range(nh):
            wt = gw.tile([128, H3], FPR, tag="w")
            nc.sync.dma_start(out=wt, in_=whhT[kc * 128:(kc + 1) * 128, :])
            lhs = hT_sb[:, kc]
            for jt in range(2):
                nc.tensor.matmul(rz_ps[:, jt], lhs, wt[:, jt * H:(jt + 1) * H],
                                 start=False, stop=(kc == nh - 1))
            nc.tensor.matmul(hn_ps, lhs, wt[:, 2 * H:3 * H],
                             start=(kc == 0), stop=(kc == nh - 1))

        # prefetch the first samples' annotations; queued after the GRU
        # weights so the (GRU-blocking) weight chunks land first.
        load_a(0)
        load_a(1)

        # elementwise gate math; sigmoid(x) = 0.5*tanh(0.5x)+0.5 so every
        # activation in the kernel stays inside the exp/tanh table set
        # (avoids ~2.7us ACT table reloads between sigmoid and exp sets).
        rzs = gs.tile([BL, 2 * H], FP)
        rz_flat = rz_ps.rearrange("p a b -> p (a b)")
        nc.vector.tensor_add(rzs, rz_flat, bih_sb[:, 0:2 * H])
        nc.vector.tensor_add(rzs, rzs, bhh_sb[:, 0:2 * H])
        rzt = gs.tile([BL, 2 * H], FP)
        nc.scalar.activation(rzt, rzs, AF.Tanh, scale=0.5)
        rz_sig = gs.tile([BL, 2 * H], FP)
        nc.any.tensor_scalar(rz_sig, rzt, scalar1=0.5, scalar2=0.5,
                             op0=ALU.mult, op1=ALU.add)
        # n = tanh(i_n + b_ihn + r*(h_n + b_hhn))
        t_hn = gs.tile([BL, H], FP)
        nc.vector.tensor_add(t_hn, hn_ps, bhh_sb[:, 2 * H:3 * H])
        nc.vector.tensor_mul(t_hn, t_hn, rz_sig[:, 0:H])
        nc.vector.tensor_add(t_hn, t_hn, in_ps)
        t_n = gs.tile([BL, H], FP)
        nc.vector.tensor_add(t_n, t_hn, bih_sb[:, 2 * H:3 * H])
        n_sb = gs.tile([BL, H], FP)
        nc.scalar.activation(n_sb, t_n, AF.Tanh)
        # h' = n + z*(h-n)
        hmn = gs.tile([BL, H], FP)
        nc.vector.tensor_sub(hmn, h_sb, n_sb)
        nc.vector.tensor_mul(hmn, hmn, rz_sig[:, H:2 * H])
        nc.vector.tensor_add(hnew_sb, n_sb, hmn)
        nc.sync.dma_start(out=h_new_o, in_=hnew_sb)

        # pq.T = Wq.T.T @ h'.T  (+ bq + ba), [j, s] layout for the tanh bias
        hnewT = gs.tile([128, 4, BL], FPR)
        for c in range(4):
            tr_ps = gps.tile([128, BL], FP, tag="trq", bufs=2)
            nc.tensor.transpose(tr_ps, hnew_sb[:, c * 128:(c + 1) * 128],
                                ident[0:BL, 0:BL])
            nc.any.tensor_copy(hnewT[:, c], tr_ps)
        for jc in range(4):
            pq_ps = gps.tile([128, BL], FP, tag="trq", bufs=2)
            for kc in range(4):
                nc.tensor.matmul(pq_ps,
                                 wqT_sb[:, kc, jc * 128:(jc + 1) * 128],
                                 hnewT[:, kc],
                                 start=(kc == 0), stop=(kc == 3))
            nc.any.tensor_scalar_add(pqT_sb[:, jc], pq_ps,
                                     bqaT_sb[:, jc:jc + 1])

    # =======================  Phase 2: attention  =======================
    ps_tr = ctx.enter_context(tc.tile_pool(name="ps_tr", bufs=2, space="PSUM"))
    ps_pa = ctx.enter_context(tc.tile_pool(name="ps_pa", bufs=3, space="PSUM"))
    ps_sc = ctx.enter_context(tc.tile_pool(name="ps_sc", bufs=1, space="PSUM"))
    ps_cx = ctx.enter_context(tc.tile_pool(name="ps_cx", bufs=1, space="PSUM"))

    # Per-sample state carried across the software pipeline: the softmax
    # tail (exp transpose + ctx matmuls) for sample s is emitted during
    # sample s+1's compute, so the ~2.5us reduce/exp latency never sits on
    # PE's in-order critical path.
    carry = {}

    def emit_ctx_tail(s):
        exp_sb, rinv = carry.pop(s)
        a_s = a_tiles[s]
        # exp.T columns for the ctx contraction (t on partitions)
        ax_ps = ps_tr.tile([128, BL * 4], FP, tag="tr")
        for tc8 in range(8):
            nc.tensor.transpose(ax_ps[:, tc8:tc8 + 1],
                                exp_sb[:, tc8 * 128:(tc8 + 1) * 128],
                                ident[0:1, 0:1])
        alT = smpool.tile([128, 8], FPR, tag="alT")
        nc.any.tensor_copy(alT, ax_ps[:, 0:8])
        # ctx_unnorm = exp @ A, then scale by 1/sum
        cx_ps = ps_cx.tile([1, H], FP, tag="cx")
        for tc8 in range(8):
            nc.tensor.matmul(cx_ps, alT[:, tc8:tc8 + 1], a_s[:, tc8, :],
                             start=(tc8 == 0), stop=(tc8 == 7))
        ctx_row = smpool.tile([1, H], FP, tag="cxr")
        nc.any.tensor_scalar_mul(ctx_row, cx_ps, rinv)
        nc.sync.dma_start(out=ctx_o[s:s + 1, :], in_=ctx_row)
        del a_tiles[s]

    for s in range(BL):
        aT_s = transpose_a(s)
        if s >= 1:
            emit_ctx_tail(s - 1)
        if s + 2 < BL:
            load_a(s + 2)

        # pa.T (+bias) -> tanh -> scores, in r-blocks of 512. The scores
        # matmul for group g is emitted after group g+1's pa matmuls so PE
        # never waits on the tanh latency.
        sc_ps = ps_sc.tile([1, T], FP, tag="sc")

        def emit_score(rb, jc, th):
            nc.tensor.matmul(sc_ps[:, rb * 512:(rb + 1) * 512],
                             vT_sb[:, jc:jc + 1], th,
                             start=(jc == 0), stop=(jc == 3))

        prev = None
        for rb in range(2):
            for jc in range(4):
                pa_ps = ps_pa.tile([128, 512], FP, tag="pa")
                for kc in range(4):
                    nc.tensor.matmul(
                        pa_ps,
                        waT_sb[:, kc, jc * 128:(jc + 1) * 128],
                        aT_s[:, kc, rb * 512:(rb + 1) * 512],
                        start=(kc == 0), stop=(kc == 3))
                th = thpool.tile([128, 512], FPR, tag="th")
                nc.scalar.activation(th, pa_ps, AF.Tanh,
                                     bias=pqT_sb[:, jc, s:s + 1])
                if prev is not None:
                    emit_score(*prev)
                prev = (rb, jc, th)
        emit_score(*prev)

        # softmax on the free dim (partition 0); no PE involvement
        negmax = smpool.tile([1, 1], FP, tag="nm")
        nc.vector.reduce_max(negmax, sc_ps, axis=AX.X, negate=True)
        exp_sb = smpool.tile([1, T], FP, tag="ex")
        ssum = smpool.tile([1, 1], FP, tag="sm")
        nc.scalar.activation(exp_sb, sc_ps, AF.Exp, bias=negmax,
                             accum_out=ssum)
        rinv = smpool.tile([1, 1], FP, tag="ri")
        nc.vector.reciprocal(rinv, ssum)
        align_row = smpool.tile([1, T], FP, tag="alr")
        nc.vector.tensor_scalar_mul(align_row, exp_sb, rinv)
        nc.sync.dma_start(out=align_o[s:s + 1, :], in_=align_row)
        carry[s] = (exp_sb, rinv)
    emit_ctx_tail(BL - 1)



# ------------------------- host side -------------------------

_NC_CACHE = None


def _get_nc():
    global _NC_CACHE
    if _NC_CACHE is None:
        _NC_CACHE = build_kernel()
    return _NC_CACHE


def make_in_maps(memory, context, rnn_state, annotations,
                 W_ih, b_ih, W_hh, b_hh, Wq, bq, Wa, ba, v):
    """Shard + lay out inputs for the 8 cores (host-side layout prep only)."""
    f32 = np.float32
    c = lambda x: np.ascontiguousarray(x, dtype=f32)
    wihT = c(W_ih.T)
    whhT = c(W_hh.T)
    wqT = c(Wq.T)
    waT = c(Wa.T)
    bqaT = c((np.asarray(bq) + np.asarray(ba)).reshape(4, 128).T)
    vT = c(np.asarray(v).reshape(4, 128).T)
    in_maps = []
    for core in range(NCORES):
        s0 = core * BL
        sl = slice(s0, s0 + BL)
        x_loc = np.concatenate([memory[sl], context[sl]], axis=1)
        in_maps.append({
            "a": c(annotations[sl]),
            "xT": c(x_loc.T),
            "hT": c(rnn_state[sl].T),
            "h_nat": c(rnn_state[sl]),
            "wihT": wihT,
            "whhT": whhT,
            "wqT": wqT,
            "waT": waT,
            "bih": c(np.broadcast_to(b_ih, (BL, H3))),
            "bhh": c(np.broadcast_to(b_hh, (BL, H3))),
            "bqaT": bqaT,
            "vT": vT,
        })
    return in_maps


def run_on_cores(in_maps, **kwargs):
    nc = _get_nc()
    return run_bass_kernel_spmd(nc, in_maps, core_ids=list(range(NCORES)),
                                **kwargs)


def kernel(memory, context, rnn_state, annotations,
           W_ih, b_ih, W_hh, b_hh, Wq, bq, Wa, ba, v):
    in_maps = make_in_maps(memory, context, rnn_state, annotations,
                           W_ih, b_ih, W_hh, b_hh, Wq, bq, Wa, ba, v)
    res = run_on_cores(in_maps).results
    h_new = np.concatenate([r["h_new_o"] for r in res], axis=0)
    ctx = np.concatenate([r["ctx_o"] for r in res], axis=0)
    align = np.concatenate([r["align_o"] for r in res], axis=0)
    return h_new, ctx, align


if __name__ == "__main__":
    nc = build_kernel()
    print("build ok")


# revision 16
# speedup vs baseline: 1.0660x; 1.0660x over previous
"""Trainium2 Bass kernel for AttentionRNN (GRU cell step + Bahdanau attention).

Contract: kernel(**inputs) takes the FULL unsharded inputs (B=64) and returns
the full outputs (h_new, ctx, align). Internally the batch is sharded across
8 NeuronCores (8 samples per core); weights are replicated. No cross-device
communication is needed.

Math (per sample):
  x  = [memory, context]                      (768,)
  gi = x @ W_ih.T + b_ih ; gh = h @ W_hh.T + b_hh
  r = sig(gi_r+gh_r); z = sig(gi_z+gh_z); n = tanh(gi_n + r*gh_n)
  h' = (1-z)*n + z*h
  pa = A @ Wa.T + ba                          (T, H)
  s  = tanh(h'@Wq.T + bq + pa) @ v            (T,)
  al = softmax(s); ctx = al @ A               (H,)

Device layout choices:
  - All linear-layer weights are fed host-pre-transposed (W.T) so the
    contraction dim lands on SBUF partitions; matmuls run as
    out[m,n] = sum_p lhsT[p,m] * rhs[p,n].
  - pa is computed TRANSPOSED (features j on partitions, time r on free dim):
    lhsT = Wa.T tile, rhs = A.T tile. A.T is produced on-chip by PE-mode
    transposes of the natural-layout A (which is itself needed for ctx).
  - tanh(pq + bq + ba + pa) is fused into one ScalarE activation with a
    per-partition bias; scores then contract j via a v-column matmul, which
    leaves scores on the free dim -> cheap free-dim softmax.
  - exp(scores) is PE-transposed into [t,1] columns; ctx_unnorm = exp @ A via
    matmuls against natural A; both ctx and align are scaled by 1/sum(exp).
  - Everything that feeds a matmul is typed float32r end-to-end (DRAM tensors,
    SBUF tiles): 1 cycle/row at N>=256 vs 4 for fp32, and the BIR verifier
    requires fp32r matmul inputs to be *produced* as fp32r. Bytes are
    identical to fp32 on the host side.
"""

import os
import sys

for _p in ("/opt/trn_rl_repo",):
    if _p not in sys.path and os.path.isdir(_p):
        sys.path.insert(0, _p)

from contextlib import ExitStack

import numpy as np

import concourse.bass as bass
import concourse.tile as tile
from concourse import bacc, mybir
from concourse.bass_utils import run_bass_kernel_spmd
from concourse.masks import make_identity

# Problem shapes (hardcoded per contract).
B, T, H, M = 64, 1024, 512, 256
NCORES = 8
BL = B // NCORES  # samples per core
KX = M + H        # GRU input width (768)
H3 = 3 * H
FP = mybir.dt.float32
FPR = mybir.dt.float32r
BF = mybir.dt.bfloat16
AF = mybir.ActivationFunctionType
ALU = mybir.AluOpType
AX = mybir.AxisListType


def build_kernel():
    """Builds and compiles the per-core Bass program. Returns nc."""
    nc = bacc.Bacc(
        "TRN2",
        target_bir_lowering=False,
        debug=False,
        enable_asserts=False,
        num_devices=NCORES,
    )

    # DRAM I/O (per core). float32r tensors carry plain fp32 bytes.
    a = nc.dram_tensor("a", [BL, T, H], FPR, kind="ExternalInput").ap()
    xT = nc.dram_tensor("xT", [KX, BL], FPR, kind="ExternalInput").ap()
    hT = nc.dram_tensor("hT", [H, BL], FPR, kind="ExternalInput").ap()
    h_nat = nc.dram_tensor("h_nat", [BL, H], FP, kind="ExternalInput").ap()
    wihT = nc.dram_tensor("wihT", [KX, H3], FPR, kind="ExternalInput").ap()
    whhT = nc.dram_tensor("whhT", [H, H3], FPR, kind="ExternalInput").ap()
    wqT = nc.dram_tensor("wqT", [H, H], FPR, kind="ExternalInput").ap()
    waT = nc.dram_tensor("waT", [H, H], FPR, kind="ExternalInput").ap()
    bih = nc.dram_tensor("bih", [BL, H3], FP, kind="ExternalInput").ap()
    bhh = nc.dram_tensor("bhh", [BL, H3], FP, kind="ExternalInput").ap()
    bqaT = nc.dram_tensor("bqaT", [128, 4], FP, kind="ExternalInput").ap()
    vT = nc.dram_tensor("vT", [128, 4], FPR, kind="ExternalInput").ap()

    h_new_o = nc.dram_tensor("h_new_o", [BL, H], FP, kind="ExternalOutput").ap()
    ctx_o = nc.dram_tensor("ctx_o", [BL, H], FP, kind="ExternalOutput").ap()
    align_o = nc.dram_tensor("align_o", [BL, T], FP, kind="ExternalOutput").ap()

    with tile.TileContext(nc) as tc, ExitStack() as ctx:
        _body(ctx, tc, a, xT, hT, h_nat, wihT, whhT, wqT, waT, bih, bhh,
              bqaT, vT, h_new_o, ctx_o, align_o)

    nc.compile()
    return nc


def _body(ctx, tc, a, xT, hT, h_nat, wihT, whhT, wqT, waT, bih, bhh,
          bqaT, vT, h_new_o, ctx_o, align_o):
    nc = tc.nc

    # ----- persistent tiles -----
    per = ctx.enter_context(tc.tile_pool(name="per", bufs=1))
    ident = per.tile([128, 128], FP)
    make_identity(nc, ident)
    # fp32r copy of the identity so A-block transposes run in fp32r mode
    # (1.5 cycles/row instead of 2).
    identR = per.tile([128, 128], FPR)
    nc.vector.tensor_copy(identR, ident)

    waT_sb = per.tile([128, 4, H], FPR)
    nc.scalar.dma_start(out=waT_sb, in_=waT.rearrange("(c p) j -> p c j", p=128))
    vT_sb = per.tile([128, 4], FPR)
    nc.scalar.dma_start(out=vT_sb, in_=vT)
    bqaT_sb = per.tile([128, 4], FP)
    nc.scalar.dma_start(out=bqaT_sb, in_=bqaT)
    # tanh bias per (j, sample): pq.T + (bq+ba).T, filled during GRU phase.
    pqT_sb = per.tile([128, 4, BL], FP)
    hnew_sb = per.tile([BL, H], FP)

    # Annotations pools are entered before the GRU phase so the first
    # samples' DMAs overlap GRU compute.
    apool = ctx.enter_context(tc.tile_pool(name="apool", bufs=3))
    atpool = ctx.enter_context(tc.tile_pool(name="atpool", bufs=2))
    thpool = ctx.enter_context(tc.tile_pool(name="thpool", bufs=4))
    smpool = ctx.enter_context(tc.tile_pool(name="smpool", bufs=2))
    a_r = a.rearrange("s (c p) d -> s p c d", p=128)
    a_tiles = {}

    def load_a(s):
        t = apool.tile([128, T // 128, H], FPR, tag="a", name=f"a{s}")
        nc.scalar.dma_start(out=t, in_=a_r[s])
        a_tiles[s] = t

    def transpose_a(s):
        aT_s = atpool.tile([128, 4, T], FPR, tag="at", name=f"at{s}")
        a_s = a_tiles[s]
        for kc in range(4):
            for half in range(2):
                tp = ps_tr.tile([128, 512], FPR, tag="tr")
                for i in range(4):
                    tci = half * 4 + i
                    nc.tensor.transpose(
                        tp[:, i * 128:(i + 1) * 128],
                        a_s[:, tci, kc * 128:(kc + 1) * 128],
                        identR)
                nc.any.tensor_copy(aT_s[:, kc, half * 512:(half + 1) * 512], tp)
        return aT_s

    # =======================  Phase 1: GRU  =======================
    with (
        tc.tile_pool(name="gw", bufs=3) as gw,
        tc.tile_pool(name="gs", bufs=1) as gs,
        tc.tile_pool(name="gps", bufs=1, space="PSUM") as gps,
    ):
        xT_sb = gs.tile([128, KX // 128, BL], FPR)
        nc.sync.dma_start(out=xT_sb, in_=xT.rearrange("(c p) s -> p c s", p=128))
        hT_sb = gs.tile([128, H // 128, BL], FPR)
        nc.sync.dma_start(out=hT_sb, in_=hT.rearrange("(c p) s -> p c s", p=128))
        h_sb = gs.tile([BL, H], FP)
        nc.sync.dma_start(out=h_sb, in_=h_nat)
        bih_sb = gs.tile([BL, H3], FP)
        nc.sync.dma_start(out=bih_sb, in_=bih)
        bhh_sb = gs.tile([BL, H3], FP)
        nc.sync.dma_start(out=bhh_sb, in_=bhh)
        wqT_sb = gs.tile([128, 4, H], FPR)

        # gates: rz_ps accumulates x@Wih.T + h@Whh.T for the r,z thirds;
        # the n third is kept split (i_n, h_n) because r gates only h_n.
        rz_ps = gps.tile([BL, 2, H], FP)     # 2 banks
        in_ps = gps.tile([BL, H], FP)        # 1 bank
        hn_ps = gps.tile([BL, H], FP)        # 1 bank
        nkx = KX // 128
        nh = H // 128
        for kc in range(nkx):
            wt = gw.tile([128, H3], FPR, tag="w")
            nc.sync.dma_start(out=wt, in_=wihT[kc * 128:(kc + 1) * 128, :])
            lhs = xT_sb[:, kc]
            for jt in range(2):
                nc.tensor.matmul(rz_ps[:, jt], lhs, wt[:, jt * H:(jt + 1) * H],
                                 start=(kc == 0), stop=False)
            nc.tensor.matmul(in_ps, lhs, wt[:, 2 * H:3 * H],
                             start=(kc == 0), stop=(kc == nkx - 1))
        for kc in range(nh):
            wt = gw.tile([128, H3], FPR, tag="w")
            nc.sync.dma_start(out=wt, in_=whhT[kc * 128:(kc + 1) * 128, :])
            lhs = hT_sb[:, kc]
            for jt in range(2):
                nc.tensor.matmul(rz_ps[:, jt], lhs, wt[:, jt * H:(jt + 1) * H],
                                 start=False, stop=(kc == nh - 1))
            nc.tensor.matmul(hn_ps, lhs, wt[:, 2 * H:3 * H],
                             start=(kc == 0), stop=(kc == nh - 1))

        nc.sync.dma_start(out=wqT_sb,
                          in_=wqT.rearrange("(c p) j -> p c j", p=128))
        # prefetch the first samples' annotations on the parallel queue
        load_a(0)
        load_a(1)

        # elementwise gate math; sigmoid(x) = 0.5*tanh(0.5x)+0.5 so every
        # activation in the kernel stays inside the exp/tanh table set
        # (avoids ~2.7us ACT table reloads between sigmoid and exp sets).
        rzs = gs.tile([BL, 2 * H], FP)
        rz_flat = rz_ps.rearrange("p a b -> p (a b)")
        nc.vector.tensor_add(rzs, rz_flat, bih_sb[:, 0:2 * H])
        nc.vector.tensor_add(rzs, rzs, bhh_sb[:, 0:2 * H])
        rzt = gs.tile([BL, 2 * H], FP)
        nc.scalar.activation(rzt, rzs, AF.Tanh, scale=0.5)
        rz_sig = gs.tile([BL, 2 * H], FP)
        nc.any.tensor_scalar(rz_sig, rzt, scalar1=0.5, scalar2=0.5,
                             op0=ALU.mult, op1=ALU.add)
        # n = tanh(i_n + b_ihn + r*(h_n + b_hhn))
        t_hn = gs.tile([BL, H], FP)
        nc.vector.tensor_add(t_hn, hn_ps, bhh_sb[:, 2 * H:3 * H])
        nc.vector.tensor_mul(t_hn, t_hn, rz_sig[:, 0:H])
        nc.vector.tensor_add(t_hn, t_hn, in_ps)
        t_n = gs.tile([BL, H], FP)
        nc.vector.tensor_add(t_n, t_hn, bih_sb[:, 2 * H:3 * H])
        n_sb = gs.tile([BL, H], FP)
        nc.scalar.activation(n_sb, t_n, AF.Tanh)
        # h' = n + z*(h-n)
        hmn = gs.tile([BL, H], FP)
        nc.vector.tensor_sub(hmn, h_sb, n_sb)
        nc.vector.tensor_mul(hmn, hmn, rz_sig[:, H:2 * H])
        nc.vector.tensor_add(hnew_sb, n_sb, hmn)
        nc.sync.dma_start(out=h_new_o, in_=hnew_sb)

        # pq.T = Wq.T.T @ h'.T  (+ bq + ba), [j, s] layout for the tanh bias
        hnewT = gs.tile([128, 4, BL], FPR)
        for c in range(4):
            tr_ps = gps.tile([128, BL], FP, tag="trq", bufs=2)
            nc.tensor.transpose(tr_ps, hnew_sb[:, c * 128:(c + 1) * 128],
                                ident[0:BL, 0:BL])
            nc.any.tensor_copy(hnewT[:, c], tr_ps)
        for jc in range(4):
            pq_ps = gps.tile([128, BL], FP, tag="trq", bufs=2)
            for kc in range(4):
                nc.tensor.matmul(pq_ps,
                                 wqT_sb[:, kc, jc * 128:(jc + 1) * 128],
                                 hnewT[:, kc],
                                 start=(kc == 0), stop=(kc == 3))
            nc.any.tensor_scalar_add(pqT_sb[:, jc], pq_ps,
                                     bqaT_sb[:, jc:jc + 1])

    # =======================  Phase 2: attention  =======================
    ps_tr = ctx.enter_context(tc.tile_pool(name="ps_tr", bufs=2, space="PSUM"))
    ps_pa = ctx.enter_context(tc.tile_pool(name="ps_pa", bufs=3, space="PSUM"))
    ps_sc = ctx.enter_context(tc.tile_pool(name="ps_sc", bufs=1, space="PSUM"))
    ps_cx = ctx.enter_context(tc.tile_pool(name="ps_cx", bufs=1, space="PSUM"))

    # Per-sample state carried across the software pipeline: the softmax
    # tail (exp transpose + ctx matmuls) for sample s is emitted during
    # sample s+1's compute, so the ~2.5us reduce/exp latency never sits on
    # PE's in-order critical path.
    carry = {}

    def emit_ctx_tail(s):
        exp_sb, rinv = carry.pop(s)
        a_s = a_tiles[s]
        # exp.T columns for the ctx contraction (t on partitions)
        ax_ps = ps_tr.tile([128, BL * 4], FP, tag="tr")
        for tc8 in range(8):
            nc.tensor.transpose(ax_ps[:, tc8:tc8 + 1],
                                exp_sb[:, tc8 * 128:(tc8 + 1) * 128],
                                ident[0:1, 0:1])
        alT = smpool.tile([128, 8], FPR, tag="alT")
        nc.any.tensor_copy(alT, ax_ps[:, 0:8])
        # ctx_unnorm = exp @ A, then scale by 1/sum
        cx_ps = ps_cx.tile([1, H], FP, tag="cx")
        for tc8 in range(8):
            nc.tensor.matmul(cx_ps, alT[:, tc8:tc8 + 1], a_s[:, tc8, :],
                             start=(tc8 == 0), stop=(tc8 == 7))
        ctx_row = smpool.tile([1, H], FP, tag="cxr")
        nc.any.tensor_scalar_mul(ctx_row, cx_ps, rinv)
        nc.sync.dma_start(out=ctx_o[s:s + 1, :], in_=ctx_row)
        del a_tiles[s]

    for s in range(BL):
        aT_s = transpose_a(s)
        if s >= 1:
            emit_ctx_tail(s - 1)
        if s + 2 < BL:
            load_a(s + 2)

        # pa.T (+bias) -> tanh -> scores, in r-blocks of 512. The scores
        # matmul for group g is emitted after group g+1's pa matmuls so PE
        # never waits on the tanh latency.
        sc_ps = ps_sc.tile([1, T], FP, tag="sc")

        def emit_score(rb, jc, th):
            nc.tensor.matmul(sc_ps[:, rb * 512:(rb + 1) * 512],
                             vT_sb[:, jc:jc + 1], th,
                             start=(jc == 0), stop=(jc == 3))

        prev = None
        for rb in range(2):
            for jc in range(4):
                pa_ps = ps_pa.tile([128, 512], FP, tag="pa")
                for kc in range(4):
                    nc.tensor.matmul(
                        pa_ps,
                        waT_sb[:, kc, jc * 128:(jc + 1) * 128],
                        aT_s[:, kc, rb * 512:(rb + 1) * 512],
                        start=(kc == 0), stop=(kc == 3))
                th = thpool.tile([128, 512], FPR, tag="th")
                nc.scalar.activation(th, pa_ps, AF.Tanh,
                                     bias=pqT_sb[:, jc, s:s + 1])
                if prev is not None:
                    emit_score(*prev)
                prev = (rb, jc, th)
        emit_score(*prev)

        # softmax on the free dim (partition 0); no PE involvement
        negmax = smpool.tile([1, 1], FP, tag="nm")
        nc.vector.reduce_max(negmax, sc_ps, axis=AX.X, negate=True)
        exp_sb = smpool.tile([1, T], FP, tag="ex")
        ssum = smpool.tile([1, 1], FP, tag="sm")
        nc.scalar.activation(exp_sb, sc_ps, AF.Exp, bias=negmax,
                             accum_out=ssum)
        rinv = smpool.tile([1, 1], FP, tag="ri")
        nc.vector.reciprocal(rinv, ssum)
        align_row = smpool.tile([1, T], FP, tag="alr")
        nc.vector.tensor_scalar_mul(align_row, exp_sb, rinv)
        nc.sync.dma_start(out=align_o[s:s + 1, :], in_=align_row)
        carry[s] = (exp_sb, rinv)
    emit_ctx_tail(BL - 1)



# ------------------------- host side -------------------------

_NC_CACHE = None


def _get_nc():
    global _NC_CACHE
    if _NC_CACHE is None:
        _NC_CACHE = build_kernel()
    return _NC_CACHE


def make_in_maps(memory, context, rnn_state, annotations,
                 W_ih, b_ih, W_hh, b_hh, Wq, bq, Wa, ba, v):
    """Shard + lay out inputs for the 8 cores (host-side layout prep only)."""
    import ml_dtypes
    f32 = np.float32
    bf16 = ml_dtypes.bfloat16
    c = lambda x: np.ascontiguousarray(x, dtype=f32)
    cb = lambda x: np.ascontiguousarray(np.asarray(x, dtype=f32), dtype=bf16)
    wihT = c(W_ih.T)
    whhT = c(W_hh.T)
    wqT = c(Wq.T)
    waT = c(Wa.T)
    bqaT = c((np.asarray(bq) + np.asarray(ba)).reshape(4, 128).T)
    vT = c(np.asarray(v).reshape(4, 128).T)
    in_maps = []
    for core in range(NCORES):
        s0 = core * BL
        sl = slice(s0, s0 + BL)
        x_loc = np.concatenate([memory[sl], context[sl]], axis=1)
        in_maps.append({
            "a": c(annotations[sl]),
            "xT": c(x_loc.T),
            "hT": c(rnn_state[sl].T),
            "h_nat": c(rnn_state[sl]),
            "wihT": wihT,
            "whhT": whhT,
            "wqT": wqT,
            "waT": waT,
            "bih": c(np.broadcast_to(b_ih, (BL, H3))),
            "bhh": c(np.broadcast_to(b_hh, (BL, H3))),
            "bqaT": bqaT,
            "vT": vT,
        })
    return in_maps


def run_on_cores(in_maps, **kwargs):
    nc = _get_nc()
    return run_bass_kernel_spmd(nc, in_maps, core_ids=list(range(NCORES)),
                                **kwargs)


def kernel(memory, context, rnn_state, annotations,
           W_ih, b_ih, W_hh, b_hh, Wq, bq, Wa, ba, v):
    in_maps = make_in_maps(memory, context, rnn_state, annotations,
                           W_ih, b_ih, W_hh, b_hh, Wq, bq, Wa, ba, v)
    res = run_on_cores(in_maps).results
    h_new = np.concatenate([r["h_new_o"] for r in res], axis=0)
    ctx = np.concatenate([r["ctx_o"] for r in res], axis=0)
    align = np.concatenate([r["align_o"] for r in res], axis=0)
    return h_new, ctx, align


if __name__ == "__main__":
    nc = build_kernel()
    print("build ok")


# revision 17
# speedup vs baseline: 1.1038x; 1.0354x over previous
"""Trainium2 Bass kernel for AttentionRNN (GRU cell step + Bahdanau attention).

Contract: kernel(**inputs) takes the FULL unsharded inputs (B=64) and returns
the full outputs (h_new, ctx, align). Internally the batch is sharded across
8 NeuronCores (8 samples per core); weights are replicated. No cross-device
communication is needed.

Math (per sample):
  x  = [memory, context]                      (768,)
  gi = x @ W_ih.T + b_ih ; gh = h @ W_hh.T + b_hh
  r = sig(gi_r+gh_r); z = sig(gi_z+gh_z); n = tanh(gi_n + r*gh_n)
  h' = (1-z)*n + z*h
  pa = A @ Wa.T + ba                          (T, H)
  s  = tanh(h'@Wq.T + bq + pa) @ v            (T,)
  al = softmax(s); ctx = al @ A               (H,)

Device layout choices:
  - All linear-layer weights are fed host-pre-transposed (W.T) so the
    contraction dim lands on SBUF partitions; matmuls run as
    out[m,n] = sum_p lhsT[p,m] * rhs[p,n].
  - pa is computed TRANSPOSED (features j on partitions, time r on free dim):
    lhsT = Wa.T tile, rhs = A.T tile. A.T is produced on-chip by PE-mode
    transposes of the natural-layout A (which is itself needed for ctx).
  - tanh(pq + bq + ba + pa) is fused into one ScalarE activation with a
    per-partition bias; scores then contract j via a v-column matmul, which
    leaves scores on the free dim -> cheap free-dim softmax.
  - exp(scores) is PE-transposed into [t,1] columns; ctx_unnorm = exp @ A via
    matmuls against natural A; both ctx and align are scaled by 1/sum(exp).
  - Everything that feeds a matmul is typed float32r end-to-end (DRAM tensors,
    SBUF tiles): 1 cycle/row at N>=256 vs 4 for fp32, and the BIR verifier
    requires fp32r matmul inputs to be *produced* as fp32r. Bytes are
    identical to fp32 on the host side.
"""

import os
import sys

for _p in ("/opt/trn_rl_repo",):
    if _p not in sys.path and os.path.isdir(_p):
        sys.path.insert(0, _p)

from contextlib import ExitStack

import numpy as np

import concourse.bass as bass
import concourse.tile as tile
from concourse import bacc, mybir
from concourse.bass_utils import run_bass_kernel_spmd
from concourse.masks import make_identity

# Problem shapes (hardcoded per contract).
B, T, H, M = 64, 1024, 512, 256
NCORES = 8
BL = B // NCORES  # samples per core
KX = M + H        # GRU input width (768)
H3 = 3 * H
FP = mybir.dt.float32
FPR = mybir.dt.float32r
BF = mybir.dt.bfloat16
AF = mybir.ActivationFunctionType
ALU = mybir.AluOpType
AX = mybir.AxisListType


def build_kernel():
    """Builds and compiles the per-core Bass program. Returns nc."""
    nc = bacc.Bacc(
        "TRN2",
        target_bir_lowering=False,
        debug=False,
        enable_asserts=False,
        num_devices=NCORES,
    )

    # DRAM I/O (per core). float32r tensors carry plain fp32 bytes.
    a = nc.dram_tensor("a", [BL, T, H], FPR, kind="ExternalInput").ap()
    xT = nc.dram_tensor("xT", [KX, BL], FPR, kind="ExternalInput").ap()
    hT = nc.dram_tensor("hT", [H, BL], FPR, kind="ExternalInput").ap()
    h_nat = nc.dram_tensor("h_nat", [BL, H], FP, kind="ExternalInput").ap()
    wihT = nc.dram_tensor("wihT", [KX, H3], FPR, kind="ExternalInput").ap()
    whhT = nc.dram_tensor("whhT", [H, H3], FPR, kind="ExternalInput").ap()
    wqT = nc.dram_tensor("wqT", [H, H], FPR, kind="ExternalInput").ap()
    waT = nc.dram_tensor("waT", [H, H], FPR, kind="ExternalInput").ap()
    bih = nc.dram_tensor("bih", [BL, H3], FP, kind="ExternalInput").ap()
    bhh = nc.dram_tensor("bhh", [BL, H3], FP, kind="ExternalInput").ap()
    bqaT = nc.dram_tensor("bqaT", [128, 4], FP, kind="ExternalInput").ap()
    vT = nc.dram_tensor("vT", [128, 4], FPR, kind="ExternalInput").ap()

    h_new_o = nc.dram_tensor("h_new_o", [BL, H], FP, kind="ExternalOutput").ap()
    ctx_o = nc.dram_tensor("ctx_o", [BL, H], FP, kind="ExternalOutput").ap()
    align_o = nc.dram_tensor("align_o", [BL, T], FP, kind="ExternalOutput").ap()

    with tile.TileContext(nc) as tc, ExitStack() as ctx:
        _body(ctx, tc, a, xT, hT, h_nat, wihT, whhT, wqT, waT, bih, bhh,
              bqaT, vT, h_new_o, ctx_o, align_o)

    nc.compile()
    return nc


def _body(ctx, tc, a, xT, hT, h_nat, wihT, whhT, wqT, waT, bih, bhh,
          bqaT, vT, h_new_o, ctx_o, align_o):
    nc = tc.nc

    # ----- persistent tiles -----
    per = ctx.enter_context(tc.tile_pool(name="per", bufs=1))
    ident = per.tile([128, 128], FP)
    make_identity(nc, ident)
    # fp32r copy of the identity so A-block transposes run in fp32r mode
    # (1.5 cycles/row instead of 2).
    identR = per.tile([128, 128], FPR)
    nc.vector.tensor_copy(identR, ident)

    waT_sb = per.tile([128, 4, H], FPR)
    nc.scalar.dma_start(out=waT_sb, in_=waT.rearrange("(c p) j -> p c j", p=128))
    vT_sb = per.tile([128, 4], FPR)
    nc.scalar.dma_start(out=vT_sb, in_=vT)
    bqaT_sb = per.tile([128, 4], FP)
    nc.scalar.dma_start(out=bqaT_sb, in_=bqaT)
    # tanh bias per (j, sample): pq.T + (bq+ba).T, filled during GRU phase.
    pqT_sb = per.tile([128, 4, BL], FP)
    hnew_sb = per.tile([BL, H], FP)

    # Annotations pools are entered before the GRU phase so the first
    # samples' DMAs overlap GRU compute.
    apool = ctx.enter_context(tc.tile_pool(name="apool", bufs=3))
    atpool = ctx.enter_context(tc.tile_pool(name="atpool", bufs=2))
    thpool = ctx.enter_context(tc.tile_pool(name="thpool", bufs=4))
    smpool = ctx.enter_context(tc.tile_pool(name="smpool", bufs=2))
    a_r = a.rearrange("s (c p) d -> s p c d", p=128)
    a_tiles = {}

    def load_a(s):
        t = apool.tile([128, T // 128, H], FPR, tag="a", name=f"a{s}")
        nc.scalar.dma_start(out=t, in_=a_r[s])
        a_tiles[s] = t

    aT_tiles = {}

    def transpose_a(s, pool=None, tag="tr"):
        aT_s = atpool.tile([128, 4, T], FPR, tag="at", name=f"at{s}")
        a_s = a_tiles[s]
        for kc in range(4):
            for half in range(2):
                tp = (pool or ps_tr).tile([128, 512], FPR, tag=tag, bufs=2)
                for i in range(4):
                    tci = half * 4 + i
                    nc.tensor.transpose(
                        tp[:, i * 128:(i + 1) * 128],
                        a_s[:, tci, kc * 128:(kc + 1) * 128],
                        identR)
                nc.any.tensor_copy(aT_s[:, kc, half * 512:(half + 1) * 512], tp)
        aT_tiles[s] = aT_s
        return aT_s

    # =======================  Phase 1: GRU  =======================
    with (
        tc.tile_pool(name="gw", bufs=3) as gw,
        tc.tile_pool(name="gs", bufs=1) as gs,
        tc.tile_pool(name="gps", bufs=1, space="PSUM") as gps,
    ):
        xT_sb = gs.tile([128, KX // 128, BL], FPR)
        nc.sync.dma_start(out=xT_sb, in_=xT.rearrange("(c p) s -> p c s", p=128))
        hT_sb = gs.tile([128, H // 128, BL], FPR)
        nc.sync.dma_start(out=hT_sb, in_=hT.rearrange("(c p) s -> p c s", p=128))
        h_sb = gs.tile([BL, H], FP)
        nc.sync.dma_start(out=h_sb, in_=h_nat)
        bih_sb = gs.tile([BL, H3], FP)
        nc.sync.dma_start(out=bih_sb, in_=bih)
        bhh_sb = gs.tile([BL, H3], FP)
        nc.sync.dma_start(out=bhh_sb, in_=bhh)
        wqT_sb = gs.tile([128, 4, H], FPR)

        # gates: rz_ps accumulates x@Wih.T + h@Whh.T for the r,z thirds;
        # the n third is kept split (i_n, h_n) because r gates only h_n.
        rz_ps = gps.tile([BL, 2, H], FP)     # 2 banks
        in_ps = gps.tile([BL, H], FP)        # 1 bank
        hn_ps = gps.tile([BL, H], FP)        # 1 bank
        nkx = KX // 128
        nh = H // 128
        for kc in range(nkx):
            wt = gw.tile([128, H3], FPR, tag="w")
            nc.sync.dma_start(out=wt, in_=wihT[kc * 128:(kc + 1) * 128, :])
            lhs = xT_sb[:, kc]
            for jt in range(2):
                nc.tensor.matmul(rz_ps[:, jt], lhs, wt[:, jt * H:(jt + 1) * H],
                                 start=(kc == 0), stop=False)
            nc.tensor.matmul(in_ps, lhs, wt[:, 2 * H:3 * H],
                             start=(kc == 0), stop=(kc == nkx - 1))
        for kc in range(nh):
            wt = gw.tile([128, H3], FPR, tag="w")
            nc.sync.dma_start(out=wt, in_=whhT[kc * 128:(kc + 1) * 128, :])
            lhs = hT_sb[:, kc]
            for jt in range(2):
                nc.tensor.matmul(rz_ps[:, jt], lhs, wt[:, jt * H:(jt + 1) * H],
                                 start=False, stop=(kc == nh - 1))
            nc.tensor.matmul(hn_ps, lhs, wt[:, 2 * H:3 * H],
                             start=(kc == 0), stop=(kc == nh - 1))

        nc.sync.dma_start(out=wqT_sb,
                          in_=wqT.rearrange("(c p) j -> p c j", p=128))
        # prefetch the first samples' annotations on the parallel queue
        load_a(0)
        load_a(1)
        # bias sum for the r,z gates: constants, computed off the critical
        # chain as soon as the bias DMAs land.
        bsum = gs.tile([BL, 2 * H], FP)
        nc.vector.tensor_add(bsum, bih_sb[:, 0:2 * H], bhh_sb[:, 0:2 * H])

        # sample-0 A.T transposes: emitted before the gate-elementwise chain
        # so PE has work while the (serial) chain runs on DVE/ACT. Uses the
        # gps "trq" slots -- no extra PSUM.
        transpose_a(0, pool=gps, tag="trq")

        # elementwise gate math; sigmoid(x) = 0.5*tanh(0.5x)+0.5 so every
        # activation in the kernel stays inside the exp/tanh table set
        # (avoids ~2.7us ACT table reloads between sigmoid and exp sets).
        # r and z are computed as separate halves: r gates the n-branch, so
        # shortening its latency shortens the whole serial chain; z is only
        # needed at the end and computes in parallel.
        r_pre = gs.tile([BL, H], FP)
        nc.vector.tensor_add(r_pre, rz_ps[:, 0], bsum[:, 0:H])
        r_t = gs.tile([BL, H], FP)
        nc.scalar.activation(r_t, r_pre, AF.Tanh, scale=0.5)
        r_sig = gs.tile([BL, H], FP)
        nc.vector.tensor_scalar(r_sig, r_t, scalar1=0.5, scalar2=0.5,
                                op0=ALU.mult, op1=ALU.add)
        # n = tanh(i_n + b_ihn + r*(h_n + b_hhn))
        t_hn = gs.tile([BL, H], FP)
        nc.vector.tensor_add(t_hn, hn_ps, bhh_sb[:, 2 * H:3 * H])
        t_in = gs.tile([BL, H], FP)
        nc.vector.tensor_add(t_in, in_ps, bih_sb[:, 2 * H:3 * H])
        nc.vector.tensor_mul(t_hn, t_hn, r_sig)
        nc.vector.tensor_add(t_hn, t_hn, t_in)
        n_sb = gs.tile([BL, H], FP)
        nc.scalar.activation(n_sb, t_hn, AF.Tanh)
        # z half, off the critical path
        z_pre = gs.tile([BL, H], FP)
        nc.vector.tensor_add(z_pre, rz_ps[:, 1], bsum[:, H:2 * H])
        z_t = gs.tile([BL, H], FP)
        nc.scalar.activation(z_t, z_pre, AF.Tanh, scale=0.5)
        z_sig = gs.tile([BL, H], FP)
        nc.vector.tensor_scalar(z_sig, z_t, scalar1=0.5, scalar2=0.5,
                                op0=ALU.mult, op1=ALU.add)
        # h' = n + z*(h-n)
        hmn = gs.tile([BL, H], FP)
        nc.vector.tensor_sub(hmn, h_sb, n_sb)
        nc.vector.tensor_mul(hmn, hmn, z_sig)
        nc.vector.tensor_add(hnew_sb, n_sb, hmn)
        nc.sync.dma_start(out=h_new_o, in_=hnew_sb)

        # pq.T = Wq.T.T @ h'.T  (+ bq + ba), [j, s] layout for the tanh bias
        hnewT = gs.tile([128, 4, BL], FPR)
        for c in range(4):
            tr_ps = gps.tile([128, BL], FP, tag="trq", bufs=2)
            nc.tensor.transpose(tr_ps, hnew_sb[:, c * 128:(c + 1) * 128],
                                ident[0:BL, 0:BL])
            nc.any.tensor_copy(hnewT[:, c], tr_ps)
        for jc in range(4):
            pq_ps = gps.tile([128, BL], FP, tag="trq", bufs=2)
            for kc in range(4):
                nc.tensor.matmul(pq_ps,
                                 wqT_sb[:, kc, jc * 128:(jc + 1) * 128],
                                 hnewT[:, kc],
                                 start=(kc == 0), stop=(kc == 3))
            nc.any.tensor_scalar_add(pqT_sb[:, jc], pq_ps,
                                     bqaT_sb[:, jc:jc + 1])

    # =======================  Phase 2: attention  =======================
    ps_tr = ctx.enter_context(tc.tile_pool(name="ps_tr", bufs=2, space="PSUM"))
    ps_pa = ctx.enter_context(tc.tile_pool(name="ps_pa", bufs=3, space="PSUM"))
    ps_sc = ctx.enter_context(tc.tile_pool(name="ps_sc", bufs=1, space="PSUM"))
    ps_cx = ctx.enter_context(tc.tile_pool(name="ps_cx", bufs=1, space="PSUM"))

    # Per-sample state carried across the software pipeline: the softmax
    # tail (exp transpose + ctx matmuls) for sample s is emitted during
    # sample s+1's compute, so the ~2.5us reduce/exp latency never sits on
    # PE's in-order critical path.
    carry = {}

    def emit_ctx_tail(s):
        exp_sb, rinv = carry.pop(s)
        a_s = a_tiles[s]
        # exp.T columns for the ctx contraction (t on partitions)
        ax_ps = ps_tr.tile([128, BL * 4], FP, tag="tr")
        for tc8 in range(8):
            nc.tensor.transpose(ax_ps[:, tc8:tc8 + 1],
                                exp_sb[:, tc8 * 128:(tc8 + 1) * 128],
                                ident[0:1, 0:1])
        alT = smpool.tile([128, 8], FPR, tag="alT")
        nc.any.tensor_copy(alT, ax_ps[:, 0:8])
        # ctx_unnorm = exp @ A, then scale by 1/sum
        cx_ps = ps_cx.tile([1, H], FP, tag="cx")
        for tc8 in range(8):
            nc.tensor.matmul(cx_ps, alT[:, tc8:tc8 + 1], a_s[:, tc8, :],
                             start=(tc8 == 0), stop=(tc8 == 7))
        ctx_row = smpool.tile([1, H], FP, tag="cxr")
        nc.any.tensor_scalar_mul(ctx_row, cx_ps, rinv)
        nc.sync.dma_start(out=ctx_o[s:s + 1, :], in_=ctx_row)
        del a_tiles[s]

    for s in range(BL):
        aT_s = aT_tiles[s] if s in aT_tiles else transpose_a(s)
        if s >= 1:
            emit_ctx_tail(s - 1)
        if s + 2 < BL:
            load_a(s + 2)

        # pa.T (+bias) -> tanh -> scores, in r-blocks of 512. The scores
        # matmul for group g is emitted after group g+1's pa matmuls so PE
        # never waits on the tanh latency.
        sc_ps = ps_sc.tile([1, T], FP, tag="sc")

        def emit_score(rb, jc, th):
            nc.tensor.matmul(sc_ps[:, rb * 512:(rb + 1) * 512],
                             vT_sb[:, jc:jc + 1], th,
                             start=(jc == 0), stop=(jc == 3))

        prev = None
        for rb in range(2):
            for jc in range(4):
                pa_ps = ps_pa.tile([128, 512], FP, tag="pa")
                for kc in range(4):
                    nc.tensor.matmul(
                        pa_ps,
                        waT_sb[:, kc, jc * 128:(jc + 1) * 128],
                        aT_s[:, kc, rb * 512:(rb + 1) * 512],
                        start=(kc == 0), stop=(kc == 3))
                th = thpool.tile([128, 512], FPR, tag="th")
                nc.scalar.activation(th, pa_ps, AF.Tanh,
                                     bias=pqT_sb[:, jc, s:s + 1])
                if prev is not None:
                    emit_score(*prev)
                prev = (rb, jc, th)
        emit_score(*prev)

        # softmax on the free dim (partition 0); no PE involvement
        negmax = smpool.tile([1, 1], FP, tag="nm")
        nc.vector.reduce_max(negmax, sc_ps, axis=AX.X, negate=True)
        exp_sb = smpool.tile([1, T], FP, tag="ex")
        ssum = smpool.tile([1, 1], FP, tag="sm")
        nc.scalar.activation(exp_sb, sc_ps, AF.Exp, bias=negmax,
                             accum_out=ssum)
        rinv = smpool.tile([1, 1], FP, tag="ri")
        nc.vector.reciprocal(rinv, ssum)
        align_row = smpool.tile([1, T], FP, tag="alr")
        nc.vector.tensor_scalar_mul(align_row, exp_sb, rinv)
        nc.sync.dma_start(out=align_o[s:s + 1, :], in_=align_row)
        carry[s] = (exp_sb, rinv)
    emit_ctx_tail(BL - 1)



# ------------------------- host side -------------------------

_NC_CACHE = None


def _get_nc():
    global _NC_CACHE
    if _NC_CACHE is None:
        _NC_CACHE = build_kernel()
    return _NC_CACHE


def make_in_maps(memory, context, rnn_state, annotations,
                 W_ih, b_ih, W_hh, b_hh, Wq, bq, Wa, ba, v):
    """Shard + lay out inputs for the 8 cores (host-side layout prep only)."""
    import ml_dtypes
    f32 = np.float32
    bf16 = ml_dtypes.bfloat16
    c = lambda x: np.ascontiguousarray(x, dtype=f32)
    cb = lambda x: np.ascontiguousarray(np.asarray(x, dtype=f32), dtype=bf16)
    wihT = c(W_ih.T)
    whhT = c(W_hh.T)
    wqT = c(Wq.T)
    waT = c(Wa.T)
    bqaT = c((np.asarray(bq) + np.asarray(ba)).reshape(4, 128).T)
    vT = c(np.asarray(v).reshape(4, 128).T)
    in_maps = []
    for core in range(NCORES):
        s0 = core * BL
        sl = slice(s0, s0 + BL)
        x_loc = np.concatenate([memory[sl], context[sl]], axis=1)
        in_maps.append({
            "a": c(annotations[sl]),
            "xT": c(x_loc.T),
            "hT": c(rnn_state[sl].T),
            "h_nat": c(rnn_state[sl]),
            "wihT": wihT,
            "whhT": whhT,
            "wqT": wqT,
            "waT": waT,
            "bih": c(np.broadcast_to(b_ih, (BL, H3))),
            "bhh": c(np.broadcast_to(b_hh, (BL, H3))),
            "bqaT": bqaT,
            "vT": vT,
        })
    return in_maps


def run_on_cores(in_maps, **kwargs):
    nc = _get_nc()
    return run_bass_kernel_spmd(nc, in_maps, core_ids=list(range(NCORES)),
                                **kwargs)


def kernel(memory, context, rnn_state, annotations,
           W_ih, b_ih, W_hh, b_hh, Wq, bq, Wa, ba, v):
    in_maps = make_in_maps(memory, context, rnn_state, annotations,
                           W_ih, b_ih, W_hh, b_hh, Wq, bq, Wa, ba, v)
    res = run_on_cores(in_maps).results
    h_new = np.concatenate([r["h_new_o"] for r in res], axis=0)
    ctx = np.concatenate([r["ctx_o"] for r in res], axis=0)
    align = np.concatenate([r["align_o"] for r in res], axis=0)
    return h_new, ctx, align


if __name__ == "__main__":
    nc = build_kernel()
    print("build ok")


# revision 19
# speedup vs baseline: 1.2267x; 1.1113x over previous
"""Trainium2 Bass kernel for AttentionRNN (GRU cell step + Bahdanau attention).

Contract: kernel(**inputs) takes the FULL unsharded inputs (B=64) and returns
the full outputs (h_new, ctx, align). Internally the batch is sharded across
8 NeuronCores (8 samples per core); weights are replicated. No cross-device
communication is needed.

Math (per sample):
  x  = [memory, context]                      (768,)
  gi = x @ W_ih.T + b_ih ; gh = h @ W_hh.T + b_hh
  r = sig(gi_r+gh_r); z = sig(gi_z+gh_z); n = tanh(gi_n + r*gh_n)
  h' = (1-z)*n + z*h
  pa = A @ Wa.T + ba                          (T, H)
  s  = tanh(h'@Wq.T + bq + pa) @ v            (T,)
  al = softmax(s); ctx = al @ A               (H,)

Device layout choices:
  - All linear-layer weights are fed host-pre-transposed (W.T) so the
    contraction dim lands on SBUF partitions; matmuls run as
    out[m,n] = sum_p lhsT[p,m] * rhs[p,n].
  - pa is computed TRANSPOSED (features j on partitions, time r on free dim):
    lhsT = Wa.T tile, rhs = A.T tile. A.T is produced on-chip by PE-mode
    transposes of the natural-layout A (which is itself needed for ctx).
  - tanh(pq + bq + ba + pa) is fused into one ScalarE activation with a
    per-partition bias; scores then contract j via a v-column matmul, which
    leaves scores on the free dim -> cheap free-dim softmax.
  - exp(scores) is PE-transposed into [t,1] columns; ctx_unnorm = exp @ A via
    matmuls against natural A; both ctx and align are scaled by 1/sum(exp).
  - Everything that feeds a matmul is typed float32r end-to-end (DRAM tensors,
    SBUF tiles): 1 cycle/row at N>=256 vs 4 for fp32, and the BIR verifier
    requires fp32r matmul inputs to be *produced* as fp32r. Bytes are
    identical to fp32 on the host side.
"""

import os
import sys

for _p in ("/opt/trn_rl_repo",):
    if _p not in sys.path and os.path.isdir(_p):
        sys.path.insert(0, _p)

from contextlib import ExitStack

import numpy as np

import concourse.bass as bass
import concourse.tile as tile
from concourse import bacc, mybir
from concourse.bass_utils import run_bass_kernel_spmd
from concourse.masks import make_identity

# Problem shapes (hardcoded per contract).
B, T, H, M = 64, 1024, 512, 256
NCORES = 8
BL = B // NCORES  # samples per core
KX = M + H        # GRU input width (768)
H3 = 3 * H
FP = mybir.dt.float32
FPR = mybir.dt.float32r
BF = mybir.dt.bfloat16
F16 = mybir.dt.float16
AF = mybir.ActivationFunctionType
ALU = mybir.AluOpType
AX = mybir.AxisListType


def build_kernel():
    """Builds and compiles the per-core Bass program. Returns nc."""
    nc = bacc.Bacc(
        "TRN2",
        target_bir_lowering=False,
        debug=False,
        enable_asserts=False,
        num_devices=NCORES,
    )

    # DRAM I/O (per core). float32r tensors carry plain fp32 bytes.
    a = nc.dram_tensor("a", [BL, T, H], FPR, kind="ExternalInput").ap()
    xT = nc.dram_tensor("xT", [KX, BL], F16, kind="ExternalInput").ap()
    hT = nc.dram_tensor("hT", [H, BL], F16, kind="ExternalInput").ap()
    h_nat = nc.dram_tensor("h_nat", [BL, H], FP, kind="ExternalInput").ap()
    wihT = nc.dram_tensor("wihT", [KX, H3], F16, kind="ExternalInput").ap()
    whhT = nc.dram_tensor("whhT", [H, H3], F16, kind="ExternalInput").ap()
    wqT = nc.dram_tensor("wqT", [H, H], FPR, kind="ExternalInput").ap()
    waT = nc.dram_tensor("waT", [H, H], FPR, kind="ExternalInput").ap()
    bih = nc.dram_tensor("bih", [BL, H3], FP, kind="ExternalInput").ap()
    bhh = nc.dram_tensor("bhh", [BL, H3], FP, kind="ExternalInput").ap()
    bqaT = nc.dram_tensor("bqaT", [128, 4], FP, kind="ExternalInput").ap()
    vT = nc.dram_tensor("vT", [128, 4], FPR, kind="ExternalInput").ap()

    h_new_o = nc.dram_tensor("h_new_o", [BL, H], FP, kind="ExternalOutput").ap()
    ctx_o = nc.dram_tensor("ctx_o", [BL, H], FP, kind="ExternalOutput").ap()
    align_o = nc.dram_tensor("align_o", [BL, T], FP, kind="ExternalOutput").ap()

    with tile.TileContext(nc) as tc, ExitStack() as ctx:
        _body(ctx, tc, a, xT, hT, h_nat, wihT, whhT, wqT, waT, bih, bhh,
              bqaT, vT, h_new_o, ctx_o, align_o)

    nc.compile()
    return nc


def _body(ctx, tc, a, xT, hT, h_nat, wihT, whhT, wqT, waT, bih, bhh,
          bqaT, vT, h_new_o, ctx_o, align_o):
    nc = tc.nc

    # ----- persistent tiles -----
    per = ctx.enter_context(tc.tile_pool(name="per", bufs=1))
    ident = per.tile([128, 128], FP)
    make_identity(nc, ident)
    # fp32r copy of the identity so A-block transposes run in fp32r mode
    # (1.5 cycles/row instead of 2).
    identR = per.tile([128, 128], FPR)
    nc.vector.tensor_copy(identR, ident)

    waT_sb = per.tile([128, 4, H], FPR)
    vT_sb = per.tile([128, 4], FPR)
    nc.scalar.dma_start(out=vT_sb, in_=vT)
    bqaT_sb = per.tile([128, 4], FP)
    nc.scalar.dma_start(out=bqaT_sb, in_=bqaT)
    # tanh bias per (j, sample): pq.T + (bq+ba).T, filled during GRU phase.
    pqT_sb = per.tile([128, 4, BL], FP)
    hnew_sb = per.tile([BL, H], FP)

    # Annotations pools are entered before the GRU phase so the first
    # samples' DMAs overlap GRU compute.
    apool = ctx.enter_context(tc.tile_pool(name="apool", bufs=3))
    atpool = ctx.enter_context(tc.tile_pool(name="atpool", bufs=2))
    thpool = ctx.enter_context(tc.tile_pool(name="thpool", bufs=4))
    smpool = ctx.enter_context(tc.tile_pool(name="smpool", bufs=2))
    a_r = a.rearrange("s (c p) d -> s p c d", p=128)
    a_tiles = {}

    def load_a(s):
        t = apool.tile([128, T // 128, H], FPR, tag="a", name=f"a{s}")
        nc.sync.dma_start(out=t, in_=a_r[s])
        a_tiles[s] = t

    aT_tiles = {}

    def transpose_a(s, pool=None, tag="tr"):
        aT_s = atpool.tile([128, 4, T], FPR, tag="at", name=f"at{s}")
        a_s = a_tiles[s]
        for kc in range(4):
            for half in range(2):
                tp = (pool or ps_tr).tile([128, 512], FPR, tag=tag, bufs=2)
                for i in range(4):
                    tci = half * 4 + i
                    nc.tensor.transpose(
                        tp[:, i * 128:(i + 1) * 128],
                        a_s[:, tci, kc * 128:(kc + 1) * 128],
                        identR)
                nc.any.tensor_copy(aT_s[:, kc, half * 512:(half + 1) * 512], tp)
        aT_tiles[s] = aT_s
        return aT_s

    # =======================  Phase 1: GRU  =======================
    with (
        tc.tile_pool(name="gw", bufs=3) as gw,
        tc.tile_pool(name="gs", bufs=1) as gs,
        tc.tile_pool(name="gps", bufs=1, space="PSUM") as gps,
    ):
        xT_sb = gs.tile([128, KX // 128, BL], F16)
        nc.sync.dma_start(out=xT_sb, in_=xT.rearrange("(c p) s -> p c s", p=128))
        hT_sb = gs.tile([128, H // 128, BL], F16)
        nc.sync.dma_start(out=hT_sb, in_=hT.rearrange("(c p) s -> p c s", p=128))
        h_sb = gs.tile([BL, H], FP)
        nc.sync.dma_start(out=h_sb, in_=h_nat)
        bih_sb = gs.tile([BL, H3], FP)
        nc.sync.dma_start(out=bih_sb, in_=bih)
        bhh_sb = gs.tile([BL, H3], FP)
        nc.sync.dma_start(out=bhh_sb, in_=bhh)
        wqT_sb = gs.tile([128, 4, H], FPR)

        # gates: rz_ps accumulates x@Wih.T + h@Whh.T for the r,z thirds;
        # the n third is kept split (i_n, h_n) because r gates only h_n.
        rz_ps = gps.tile([BL, 2, H], FP)     # 2 banks
        in_ps = gps.tile([BL, H], FP)        # 1 bank
        hn_ps = gps.tile([BL, H], FP)        # 1 bank
        nkx = KX // 128
        nh = H // 128
        for kc in range(nkx):
            wt = gw.tile([128, H3], F16, tag="w")
            nc.sync.dma_start(out=wt, in_=wihT[kc * 128:(kc + 1) * 128, :])
            lhs = xT_sb[:, kc]
            for jt in range(2):
                nc.tensor.matmul(rz_ps[:, jt], lhs, wt[:, jt * H:(jt + 1) * H],
                                 start=(kc == 0), stop=False)
            nc.tensor.matmul(in_ps, lhs, wt[:, 2 * H:3 * H],
                             start=(kc == 0), stop=(kc == nkx - 1))
        for kc in range(nh):
            wt = gw.tile([128, H3], F16, tag="w")
            nc.sync.dma_start(out=wt, in_=whhT[kc * 128:(kc + 1) * 128, :])
            lhs = hT_sb[:, kc]
            for jt in range(2):
                nc.tensor.matmul(rz_ps[:, jt], lhs, wt[:, jt * H:(jt + 1) * H],
                                 start=False, stop=(kc == nh - 1))
            nc.tensor.matmul(hn_ps, lhs, wt[:, 2 * H:3 * H],
                             start=(kc == 0), stop=(kc == nh - 1))

        nc.sync.dma_start(out=wqT_sb,
                          in_=wqT.rearrange("(c p) j -> p c j", p=128))
        # post-weight DMA order: everything sample-0 needs, then sample 1.
        load_a(0)
        nc.sync.dma_start(out=waT_sb,
                          in_=waT.rearrange("(c p) j -> p c j", p=128))
        load_a(1)
        # bias sum for the r,z gates: constants, computed off the critical
        # chain as soon as the bias DMAs land.
        bsum = gs.tile([BL, 2 * H], FP)
        nc.vector.tensor_add(bsum, bih_sb[:, 0:2 * H], bhh_sb[:, 0:2 * H])

        # sample-0 A.T transposes: emitted before the gate-elementwise chain
        # so PE has work while the (serial) chain runs on DVE/ACT. Uses the
        # gps "trq" slots -- no extra PSUM.
        transpose_a(0, pool=gps, tag="trq")

        # elementwise gate math; sigmoid(x) = 0.5*tanh(0.5x)+0.5 so every
        # activation in the kernel stays inside the exp/tanh table set
        # (avoids ~2.7us ACT table reloads between sigmoid and exp sets).
        # r and z are computed as separate halves: r gates the n-branch, so
        # shortening its latency shortens the whole serial chain; z is only
        # needed at the end and computes in parallel.
        r_pre = gs.tile([BL, H], FP)
        nc.vector.tensor_add(r_pre, rz_ps[:, 0], bsum[:, 0:H])
        r_t = gs.tile([BL, H], FP)
        nc.scalar.activation(r_t, r_pre, AF.Tanh, scale=0.5)
        r_sig = gs.tile([BL, H], FP)
        nc.vector.tensor_scalar(r_sig, r_t, scalar1=0.5, scalar2=0.5,
                                op0=ALU.mult, op1=ALU.add)
        # n = tanh(i_n + b_ihn + r*(h_n + b_hhn))
        t_hn = gs.tile([BL, H], FP)
        nc.vector.tensor_add(t_hn, hn_ps, bhh_sb[:, 2 * H:3 * H])
        t_in = gs.tile([BL, H], FP)
        nc.vector.tensor_add(t_in, in_ps, bih_sb[:, 2 * H:3 * H])
        nc.vector.tensor_mul(t_hn, t_hn, r_sig)
        nc.vector.tensor_add(t_hn, t_hn, t_in)
        n_sb = gs.tile([BL, H], FP)
        nc.scalar.activation(n_sb, t_hn, AF.Tanh)
        # z half, off the critical path
        z_pre = gs.tile([BL, H], FP)
        nc.vector.tensor_add(z_pre, rz_ps[:, 1], bsum[:, H:2 * H])
        z_t = gs.tile([BL, H], FP)
        nc.scalar.activation(z_t, z_pre, AF.Tanh, scale=0.5)
        z_sig = gs.tile([BL, H], FP)
        nc.vector.tensor_scalar(z_sig, z_t, scalar1=0.5, scalar2=0.5,
                                op0=ALU.mult, op1=ALU.add)
        # h' = n + z*(h-n)
        hmn = gs.tile([BL, H], FP)
        nc.vector.tensor_sub(hmn, h_sb, n_sb)
        nc.vector.tensor_mul(hmn, hmn, z_sig)
        nc.vector.tensor_add(hnew_sb, n_sb, hmn)
        nc.sync.dma_start(out=h_new_o, in_=hnew_sb)

        # pq.T = Wq.T.T @ h'.T  (+ bq + ba), [j, s] layout for the tanh bias
        hnewT = gs.tile([128, 4, BL], FPR)
        for c in range(4):
            tr_ps = gps.tile([128, BL], FP, tag="trq", bufs=2)
            nc.tensor.transpose(tr_ps, hnew_sb[:, c * 128:(c + 1) * 128],
                                ident[0:BL, 0:BL])
            nc.any.tensor_copy(hnewT[:, c], tr_ps)
        for jc in range(4):
            pq_ps = gps.tile([128, BL], FP, tag="trq", bufs=2)
            for kc in range(4):
                nc.tensor.matmul(pq_ps,
                                 wqT_sb[:, kc, jc * 128:(jc + 1) * 128],
                                 hnewT[:, kc],
                                 start=(kc == 0), stop=(kc == 3))
            nc.any.tensor_scalar_add(pqT_sb[:, jc], pq_ps,
                                     bqaT_sb[:, jc:jc + 1])

    # =======================  Phase 2: attention  =======================
    ps_tr = ctx.enter_context(tc.tile_pool(name="ps_tr", bufs=2, space="PSUM"))
    ps_pa = ctx.enter_context(tc.tile_pool(name="ps_pa", bufs=3, space="PSUM"))
    ps_sc = ctx.enter_context(tc.tile_pool(name="ps_sc", bufs=1, space="PSUM"))
    ps_cx = ctx.enter_context(tc.tile_pool(name="ps_cx", bufs=1, space="PSUM"))

    # Per-sample state carried across the software pipeline: the softmax
    # tail (exp transpose + ctx matmuls) for sample s is emitted during
    # sample s+1's compute, so the ~2.5us reduce/exp latency never sits on
    # PE's in-order critical path.
    carry = {}

    def emit_ctx_tail(s):
        exp_sb, rinv = carry.pop(s)
        a_s = a_tiles[s]
        # exp.T columns for the ctx contraction (t on partitions)
        ax_ps = ps_tr.tile([128, BL * 4], FP, tag="tr")
        for tc8 in range(8):
            nc.tensor.transpose(ax_ps[:, tc8:tc8 + 1],
                                exp_sb[:, tc8 * 128:(tc8 + 1) * 128],
                                ident[0:1, 0:1])
        alT = smpool.tile([128, 8], FPR, tag="alT")
        nc.any.tensor_copy(alT, ax_ps[:, 0:8])
        # ctx_unnorm = exp @ A, then scale by 1/sum
        cx_ps = ps_cx.tile([1, H], FP, tag="cx")
        for tc8 in range(8):
            nc.tensor.matmul(cx_ps, alT[:, tc8:tc8 + 1], a_s[:, tc8, :],
                             start=(tc8 == 0), stop=(tc8 == 7))
        ctx_row = smpool.tile([1, H], FP, tag="cxr")
        nc.any.tensor_scalar_mul(ctx_row, cx_ps, rinv)
        nc.sync.dma_start(out=ctx_o[s:s + 1, :], in_=ctx_row)
        del a_tiles[s]

    for s in range(BL):
        aT_s = aT_tiles[s] if s in aT_tiles else transpose_a(s)
        if s >= 1:
            emit_ctx_tail(s - 1)
        if s + 2 < BL:
            load_a(s + 2)

        # pa.T (+bias) -> tanh -> scores, in r-blocks of 512. The scores
        # matmul for group g is emitted after group g+1's pa matmuls so PE
        # never waits on the tanh latency.
        sc_ps = ps_sc.tile([1, T], FP, tag="sc")

        def emit_score(rb, jc, th):
            nc.tensor.matmul(sc_ps[:, rb * 512:(rb + 1) * 512],
                             vT_sb[:, jc:jc + 1], th,
                             start=(jc == 0), stop=(jc == 3))

        prev = None
        for rb in range(2):
            for jc in range(4):
                pa_ps = ps_pa.tile([128, 512], FP, tag="pa")
                for kc in range(4):
                    nc.tensor.matmul(
                        pa_ps,
                        waT_sb[:, kc, jc * 128:(jc + 1) * 128],
                        aT_s[:, kc, rb * 512:(rb + 1) * 512],
                        start=(kc == 0), stop=(kc == 3))
                th = thpool.tile([128, 512], FPR, tag="th")
                nc.scalar.activation(th, pa_ps, AF.Tanh,
                                     bias=pqT_sb[:, jc, s:s + 1])
                if prev is not None:
                    emit_score(*prev)
                prev = (rb, jc, th)
        emit_score(*prev)

        # softmax on the free dim (partition 0); no PE involvement
        negmax = smpool.tile([1, 1], FP, tag="nm")
        nc.vector.reduce_max(negmax, sc_ps, axis=AX.X, negate=True)
        exp_sb = smpool.tile([1, T], FP, tag="ex")
        ssum = smpool.tile([1, 1], FP, tag="sm")
        nc.scalar.activation(exp_sb, sc_ps, AF.Exp, bias=negmax,
                             accum_out=ssum)
        rinv = smpool.tile([1, 1], FP, tag="ri")
        nc.vector.reciprocal(rinv, ssum)
        align_row = smpool.tile([1, T], FP, tag="alr")
        nc.vector.tensor_scalar_mul(align_row, exp_sb, rinv)
        nc.sync.dma_start(out=align_o[s:s + 1, :], in_=align_row)
        carry[s] = (exp_sb, rinv)
    emit_ctx_tail(BL - 1)



# ------------------------- host side -------------------------

_NC_CACHE = None


def _get_nc():
    global _NC_CACHE
    if _NC_CACHE is None:
        _NC_CACHE = build_kernel()
    return _NC_CACHE


def make_in_maps(memory, context, rnn_state, annotations,
                 W_ih, b_ih, W_hh, b_hh, Wq, bq, Wa, ba, v):
    """Shard + lay out inputs for the 8 cores (host-side layout prep only)."""
    f32 = np.float32
    c = lambda x: np.ascontiguousarray(x, dtype=f32)
    cb = lambda x: np.ascontiguousarray(np.asarray(x, dtype=f32),
                                        dtype=np.float16)
    wihT = cb(W_ih.T)
    whhT = cb(W_hh.T)
    wqT = c(Wq.T)
    waT = c(Wa.T)
    bqaT = c((np.asarray(bq) + np.asarray(ba)).reshape(4, 128).T)
    vT = c(np.asarray(v).reshape(4, 128).T)
    in_maps = []
    for core in range(NCORES):
        s0 = core * BL
        sl = slice(s0, s0 + BL)
        x_loc = np.concatenate([memory[sl], context[sl]], axis=1)
        in_maps.append({
            "a": c(annotations[sl]),
            "xT": cb(x_loc.T),
            "hT": cb(rnn_state[sl].T),
            "h_nat": c(rnn_state[sl]),
            "wihT": wihT,
            "whhT": whhT,
            "wqT": wqT,
            "waT": waT,
            "bih": c(np.broadcast_to(b_ih, (BL, H3))),
            "bhh": c(np.broadcast_to(b_hh, (BL, H3))),
            "bqaT": bqaT,
            "vT": vT,
        })
    return in_maps


def run_on_cores(in_maps, **kwargs):
    nc = _get_nc()
    return run_bass_kernel_spmd(nc, in_maps, core_ids=list(range(NCORES)),
                                **kwargs)


def kernel(memory, context, rnn_state, annotations,
           W_ih, b_ih, W_hh, b_hh, Wq, bq, Wa, ba, v):
    in_maps = make_in_maps(memory, context, rnn_state, annotations,
                           W_ih, b_ih, W_hh, b_hh, Wq, bq, Wa, ba, v)
    res = run_on_cores(in_maps).results
    h_new = np.concatenate([r["h_new_o"] for r in res], axis=0)
    ctx = np.concatenate([r["ctx_o"] for r in res], axis=0)
    align = np.concatenate([r["align_o"] for r in res], axis=0)
    return h_new, ctx, align


if __name__ == "__main__":
    nc = build_kernel()
    print("build ok")
